# revision 7
# baseline (speedup 1.0000x reference)
"""Trainium2 Bass kernel for DND kNN retrieval (nn_DND_49022756716937).

Capacity-sharded exact kNN (k=50) + inverse-distance weighting on 8 cores.

Per core (core c owns keys [c*12500, (c+1)*12500), padded to 12800):
 - score s = 2*q.k - ||k||^2 over its shard for all 2048 queries
   (PE matmul, 16 query-tiles x 25 chunks of 512)
 - per 512-chunk top-8 (value, position) via DVE max8/max_index: any
   global-top-50 member has <8 better keys in its chunk w.p. ~1-1e-9
 - AllToAll routes candidates so core i gets query-tiles i and i+8
 - merge: 7 rounds max8/match_replace -> top-56 (s, slot); t_b = 50th s;
   winner key-ids and values via per-winner indirect DMA; weights
   w = 1/(qsq - s + 1e-3), masked s >= t_b
 - out = sum(w*v)/sum(w) -> [256, 1] per core; host reassembles.
"""
import numpy as np
import sys

sys.path.insert(0, "/opt/trn_rl_repo")

import jax
import concourse.bass as bass
from concourse import bacc
import concourse.mybir as mybir
from concourse.tile import TileContext

F32 = mybir.dt.float32
U16 = mybir.dt.uint16
U32 = mybir.dt.uint32
AF = mybir.ActivationFunctionType
ALU = mybir.AluOpType

NCORES = 8
B = 2048
D = 128
CAP = 100000
SHARD = CAP // NCORES          # 12500
PADSH = 12800                  # 25 chunks of 512
CHUNK = 512
NCH = PADSH // CHUNK           # 25
QT = B // 128                  # 16
NCAND = NCH * 8                # 200 candidates/query/core
NMERGE = NCORES * NCAND        # 1600
K = 50
NROUND = 7                     # 7*8 = 56 extracted
NSEL = NROUND * 8
DELTA = 1e-3
PAD_KEY = 1000.0
NEG = -1.0e30


def _build():
    nc = bacc.Bacc('TRN2', target_bir_lowering=False, debug=False,
                   num_devices=NCORES)

    kT = nc.dram_tensor("kT", [D, PADSH], F32, kind="ExternalInput")
    qT = nc.dram_tensor("qT", [D, B], F32, kind="ExternalInput")
    qrows = nc.dram_tensor("qrows", [256, D], F32, kind="ExternalInput")
    vals_glob = nc.dram_tensor("vals_glob", [NCORES * PADSH, 1], F32,
                               kind="ExternalInput")
    base_pat = nc.dram_tensor("base_pat", [128, NCAND], F32,
                              kind="ExternalInput")
    rowbase = nc.dram_tensor("rowbase", [128, 1], F32, kind="ExternalInput")
    y = nc.dram_tensor("y", [256, 1], F32, kind="ExternalOutput")

    with TileContext(nc) as tc:
        with tc.tile_pool(name="const", bufs=1) as cpool, \
             tc.tile_pool(name="work", bufs=6) as spool, \
             tc.tile_pool(name="cand", bufs=3) as candp, \
             tc.tile_pool(name="merge", bufs=1) as mpool, \
             tc.tile_pool(name="psum", bufs=6, space="PSUM") as psum, \
             tc.tile_pool(name="psum1", bufs=2, space="PSUM") as psum1, \
             tc.tile_pool(name="dram", bufs=1, space="DRAM") as dram:

            # ---------------- setup ----------------
            kT_t = cpool.tile([D, PADSH], F32)
            nc.sync.dma_start(out=kT_t[:], in_=kT[:])
            q2T = cpool.tile([D, B], F32)
            nc.sync.dma_start(out=q2T[:], in_=qT[:])
            nc.vector.tensor_scalar_mul(q2T[:], q2T[:], 2.0)

            ones1 = cpool.tile([1, 128], F32)
            nc.vector.memset(ones1[:], 1.0)
            negones = cpool.tile([128, 1], F32)
            nc.vector.memset(negones[:], -1.0)

            ksqn = cpool.tile([1, PADSH], F32)    # -||k||^2 row
            for c in range(NCH):
                sl = slice(c * CHUNK, (c + 1) * CHUNK)
                kk = spool.tile([D, CHUNK], F32, tag="kk")
                nc.vector.tensor_tensor(out=kk[:], in0=kT_t[:, sl],
                                        in1=kT_t[:, sl], op=ALU.mult)
                pk = psum1.tile([1, CHUNK], F32)
                nc.tensor.matmul(out=pk[:], lhsT=negones[:], rhs=kk[:],
                                 start=True, stop=True)
                nc.scalar.activation(out=ksqn[:, sl], in_=pk[:], func=AF.Copy)

            qsq_d = cpool.tile([128, 2], F32)
            for h in range(2):
                qr = spool.tile([128, D], F32, tag="qr")
                nc.sync.dma_start(out=qr[:], in_=qrows[h * 128:(h + 1) * 128, :])
                nc.scalar.activation(out=qr[:], in_=qr[:], func=AF.Square,
                                     accum_out=qsq_d[:, h:h + 1])
            nc.vector.tensor_scalar_add(qsq_d[:], qsq_d[:], DELTA)

            basep = cpool.tile([128, NCAND], F32)
            nc.sync.dma_start(out=basep[:], in_=base_pat[:])
            rowb = cpool.tile([128, 1], F32)
            nc.sync.dma_start(out=rowb[:], in_=rowbase[:])

            ag_in = dram.tile([B, 2 * NCAND], F32)
            ag_out = dram.tile([B, 2 * NCAND], F32)
            gidx_flat = dram.tile([128 * NMERGE, 1], F32)

            # ---------------- scan ----------------
            for qt in range(QT):
                cs = candp.tile([128, NCAND], F32, tag="cs")
                cp = candp.tile([128, NCAND], U16, tag="cp")
                for c in range(NCH):
                    p = psum.tile([128, CHUNK], F32)
                    nc.tensor.matmul(
                        out=p[:], lhsT=q2T[:, qt * 128:(qt + 1) * 128],
                        rhs=kT_t[:, c * CHUNK:(c + 1) * CHUNK],
                        start=True, stop=False)
                    nc.tensor.matmul(
                        out=p[:], lhsT=ones1[:],
                        rhs=ksqn[:, c * CHUNK:(c + 1) * CHUNK],
                        start=False, stop=True)
                    st = spool.tile([128, CHUNK], F32, tag="evict")
                    nc.scalar.activation(out=st[:], in_=p[:], func=AF.Copy)
                    nc.vector.max(out=cs[:, c * 8:(c + 1) * 8], in_=st[:])
                    nc.vector.max_index(out=cp[:, c * 8:(c + 1) * 8],
                                        in_max=cs[:, c * 8:(c + 1) * 8],
                                        in_values=st[:])
                gi = candp.tile([128, NCAND], F32, tag="gi")
                nc.vector.tensor_copy(gi[:], cp[:])       # u16 -> f32
                nc.vector.tensor_add(gi[:], gi[:], basep[:])
                # stage into AllToAll input: block j=(qt%8), half (qt//8)
                r0 = (qt % 8) * 256 + (qt // 8) * 128
                nc.sync.dma_start(out=ag_in[r0:r0 + 128, 0:NCAND], in_=cs[:])
                nc.sync.dma_start(out=ag_in[r0:r0 + 128, NCAND:2 * NCAND],
                                  in_=gi[:])

            nc.gpsimd.collective_compute(
                "AllToAll", ALU.bypass,
                replica_groups=[list(range(NCORES))],
                ins=[ag_in.opt()], outs=[ag_out.opt()],
            )

            # ---------------- merge (2 halves of 128 queries) -------------
            for m in range(2):
                # gather [128, 8 blocks x 200] from ag_out
                src = ag_out[:].rearrange("(blk p) c -> p blk c", p=256)
                smg = mpool.tile([128, NMERGE], F32, tag=f"smg{m}")
                gmg = mpool.tile([128, NMERGE], F32, tag=f"gmg{m}")
                off = m * 128
                nc.sync.dma_start(
                    out=smg[:].rearrange("p (blk j) -> p blk j", blk=NCORES),
                    in_=src[off:off + 128, :, 0:NCAND])
                nc.sync.dma_start(
                    out=gmg[:].rearrange("p (blk j) -> p blk j", blk=NCORES),
                    in_=src[off:off + 128, :, NCAND:2 * NCAND])
                # stage gidx (p-major flat) for slot-indexed row gather
                nc.sync.dma_start(
                    out=gidx_flat[:].rearrange("(p s) one -> p (s one)", p=128),
                    in_=gmg[:])

                work = mpool.tile([128, NMERGE], F32, tag=f"wk{m}")
                nc.vector.tensor_copy(work[:], smg[:])
                sel = mpool.tile([128, NSEL], F32, tag=f"sel{m}")
                slot = mpool.tile([128, NSEL], U32, tag=f"slot{m}")
                for r in range(NROUND):
                    s8 = sel[:, r * 8:(r + 1) * 8]
                    nc.vector.max(out=s8, in_=work[:])
                    nc.vector.max_index(out=slot[:, r * 8:(r + 1) * 8],
                                        in_max=s8, in_values=smg[:])
                    if r + 1 < NROUND:
                        nc.vector.match_replace(out=work[:], in_to_replace=s8,
                                                in_values=work[:],
                                                imm_value=NEG)

                # winner rows in gidx_flat: row = p*1600 + slot
                slotf = mpool.tile([128, NSEL], F32, tag=f"slotf{m}")
                nc.vector.tensor_copy(slotf[:], slot[:])   # u32 -> f32
                nc.vector.tensor_scalar(slotf[:], slotf[:], rowb[:, 0:1], None,
                                        op0=ALU.add)
                rowf = mpool.tile([128, NSEL], U32, tag=f"rowf{m}")
                nc.vector.tensor_copy(rowf[:], slotf[:])   # f32 -> u32
                giw = mpool.tile([128, NSEL], F32, tag=f"giw{m}")
                for j in range(NSEL):
                    nc.gpsimd.indirect_dma_start(
                        out=giw[:, j:j + 1], out_offset=None,
                        in_=gidx_flat[:],
                        in_offset=bass.IndirectOffsetOnAxis(
                            ap=rowf[:, j:j + 1], axis=0))
                giwu = mpool.tile([128, NSEL], U32, tag=f"giwu{m}")
                nc.vector.tensor_copy(giwu[:], giw[:])    # f32 -> u32
                vw = mpool.tile([128, NSEL], F32, tag=f"vw{m}")
                for j in range(NSEL):
                    nc.gpsimd.indirect_dma_start(
                        out=vw[:, j:j + 1], out_offset=None,
                        in_=vals_glob[:],
                        in_offset=bass.IndirectOffsetOnAxis(
                            ap=giwu[:, j:j + 1], axis=0))

                # weights and output
                dd = mpool.tile([128, NSEL], F32, tag=f"dd{m}")
                nc.vector.tensor_scalar(dd[:], sel[:], qsq_d[:, m:m + 1], -1.0,
                                        op0=ALU.subtract, op1=ALU.mult)
                w = mpool.tile([128, NSEL], F32, tag=f"w{m}")
                nc.vector.reciprocal(w[:], dd[:])
                msk = mpool.tile([128, NSEL], F32, tag=f"msk{m}")
                nc.vector.tensor_scalar(msk[:], sel[:], sel[:, K - 1:K], None,
                                        op0=ALU.is_ge)
                nc.vector.tensor_tensor(out=w[:], in0=w[:], in1=msk[:],
                                        op=ALU.mult)
                wv = mpool.tile([128, NSEL], F32, tag=f"wv{m}")
                nc.vector.tensor_tensor(out=wv[:], in0=w[:], in1=vw[:],
                                        op=ALU.mult)
                num = mpool.tile([128, 1], F32, tag=f"num{m}")
                den = mpool.tile([128, 1], F32, tag=f"den{m}")
                nc.vector.reduce_sum(out=num[:], in_=wv[:],
                                     axis=mybir.AxisListType.X)
                nc.vector.reduce_sum(out=den[:], in_=w[:],
                                     axis=mybir.AxisListType.X)
                nc.vector.reciprocal(den[:], den[:])
                nc.vector.tensor_tensor(out=num[:], in0=num[:], in1=den[:],
                                        op=ALU.mult)
                nc.sync.dma_start(out=y[m * 128:(m + 1) * 128, :], in_=num[:])

    nc.compile()
    return nc


class _Runner:
    """Stable jitted PJRT runner (jit built once -> warm calls are fast)."""

    def __init__(self, nc, n_cores):
        from jax.sharding import Mesh, PartitionSpec
        from jax.experimental.shard_map import shard_map
        from concourse.bass2jax import (_bass_exec_p, install_neuronx_cc_hook,
                                        partition_id_tensor)
        install_neuronx_cc_hook()
        self.n_cores = n_cores
        pname = nc.partition_id_tensor.name if nc.partition_id_tensor else None
        in_names, out_names, out_avals, zero_outs = [], [], [], []
        for alloc in nc.m.functions[0].allocations:
            if not isinstance(alloc, mybir.MemoryLocationSet):
                continue
            name = alloc.memorylocations[0].name
            if alloc.kind == "ExternalInput":
                if name != pname:
                    in_names.append(name)
            elif alloc.kind == "ExternalOutput":
                out_names.append(name)
                shape = tuple(alloc.tensor_shape)
                dtype = mybir.dt.np(alloc.dtype)
                out_avals.append(jax.core.ShapedArray(shape, dtype))
                zero_outs.append(np.zeros(shape, dtype))
        self.in_names = in_names
        self.out_names = out_names
        self.zero_outs = zero_outs
        n_params, n_outs = len(in_names), len(out_names)
        all_in = in_names + out_names + ([pname] if pname else [])

        def _body(*args):
            operands = list(args)
            if pname is not None:
                operands.append(partition_id_tensor())
            return tuple(_bass_exec_p.bind(
                *operands, out_avals=tuple(out_avals), in_names=tuple(all_in),
                out_names=tuple(out_names), lowering_input_output_aliases=(),
                sim_require_finite=True, sim_require_nnan=True, nc=nc))

        devices = jax.devices()[:n_cores]
        mesh = Mesh(np.asarray(devices), ("core",))
        self.fn = jax.jit(shard_map(
            _body, mesh=mesh,
            in_specs=(PartitionSpec("core"),) * (n_params + n_outs),
            out_specs=(PartitionSpec("core"),) * n_outs, check_rep=False))

    def __call__(self, in_maps):
        args = [np.concatenate([np.asarray(m[n]) for m in in_maps], axis=0)
                for n in self.in_names]
        args += [np.concatenate([z] * self.n_cores, axis=0)
                 for z in self.zero_outs]
        outs = [np.asarray(o) for o in self.fn(*args)]
        res = []
        for c in range(self.n_cores):
            d = {}
            for n, o in zip(self.out_names, outs):
                per = o.shape[0] // self.n_cores
                d[n] = o[c * per:(c + 1) * per]
            res.append(d)
        return res


_CACHE = {}


def _prep_inputs(queries, dnd_keys, dnd_values):
    """Shard / lay out the inputs for the 8 cores (host-side data movement)."""
    q = np.ascontiguousarray(queries, dtype=np.float32)
    kk = np.ascontiguousarray(dnd_keys, dtype=np.float32)
    vv = np.ascontiguousarray(dnd_values, dtype=np.float32).reshape(-1)

    qT = np.ascontiguousarray(q.T)                       # [128, 2048]
    vals_glob = np.zeros((NCORES * PADSH, 1), np.float32)
    in_maps = []
    for c in range(NCORES):
        ksl = kk[c * SHARD:(c + 1) * SHARD]              # [12500, 128]
        kT_p = np.full((D, PADSH), PAD_KEY, np.float32)
        kT_p[:, :SHARD] = ksl.T
        vals_glob[c * PADSH:c * PADSH + SHARD, 0] = vv[c * SHARD:(c + 1) * SHARD]
        base = (np.arange(NCAND, dtype=np.float32) // 8).astype(np.float32) \
            * CHUNK + c * PADSH
        base_pat = np.broadcast_to(base, (128, NCAND)).astype(np.float32)
        qrows_c = np.concatenate([q[c * 128:(c + 1) * 128],
                                  q[(c + 8) * 128:(c + 9) * 128]], axis=0)
        rowbase = (np.arange(128, dtype=np.float32) * NMERGE)[:, None]
        in_maps.append({
            "kT": kT_p, "qT": qT, "qrows": qrows_c,
            "base_pat": np.ascontiguousarray(base_pat),
            "rowbase": np.ascontiguousarray(rowbase),
        })
    for c in range(NCORES):
        in_maps[c]["vals_glob"] = vals_glob
    return in_maps


def kernel(queries, dnd_keys, dnd_values):
    if "run" not in _CACHE:
        nc = _build()
        _CACHE["run"] = _Runner(nc, NCORES)
    run = _CACHE["run"]
    in_maps = _prep_inputs(queries, dnd_keys, dnd_values)
    results = run(in_maps)
    out = np.zeros((B, 1), np.float32)
    for c in range(NCORES):
        yc = results[c]["y"]
        out[c * 128:(c + 1) * 128] = yc[0:128]
        out[(c + 8) * 128:(c + 9) * 128] = yc[128:256]
    return out


# revision 8
# speedup vs baseline: 1.0611x; 1.0611x over previous
"""Trainium2 Bass kernel for DND kNN retrieval (nn_DND_49022756716937).

Capacity-sharded exact kNN (k=50) + inverse-distance weighting on 8 cores.

Per core (core c owns keys [c*12500, (c+1)*12500), padded to 12800):
 - score s = 2*q.k - ||k||^2 over its shard for all 2048 queries
   (PE matmul, 16 query-tiles x 25 chunks of 512)
 - per 512-chunk top-8 (value, position) via DVE max8/max_index: any
   global-top-50 member has <8 better keys in its chunk w.p. ~1-1e-9
 - AllToAll routes candidates so core i gets query-tiles i and i+8
 - merge: 7 rounds max8/match_replace -> top-56 (s, slot); t_b = 50th s;
   winner key-ids and values via per-winner indirect DMA; weights
   w = 1/(qsq - s + 1e-3), masked s >= t_b
 - out = sum(w*v)/sum(w) -> [256, 1] per core; host reassembles.
"""
import numpy as np
import sys

sys.path.insert(0, "/opt/trn_rl_repo")

import jax
import concourse.bass as bass
from concourse import bacc
import concourse.mybir as mybir
from concourse.tile import TileContext

F32 = mybir.dt.float32
U16 = mybir.dt.uint16
U32 = mybir.dt.uint32
AF = mybir.ActivationFunctionType
ALU = mybir.AluOpType

NCORES = 8
B = 2048
D = 128
CAP = 100000
SHARD = CAP // NCORES          # 12500
PADSH = 12800                  # 25 chunks of 512
CHUNK = 512
NCH = PADSH // CHUNK           # 25
QT = B // 128                  # 16
NCAND = NCH * 8                # 200 candidates/query/core
NMERGE = NCORES * NCAND        # 1600
K = 50
NROUND = 7                     # 7*8 = 56 extracted
NSEL = NROUND * 8
DELTA = 1e-3
PAD_KEY = 1000.0
NEG = -1.0e30


def _build():
    nc = bacc.Bacc('TRN2', target_bir_lowering=False, debug=False,
                   num_devices=NCORES)

    kT = nc.dram_tensor("kT", [D, PADSH], F32, kind="ExternalInput")
    qT = nc.dram_tensor("qT", [D, B], F32, kind="ExternalInput")
    qrows = nc.dram_tensor("qrows", [256, D], F32, kind="ExternalInput")
    vals_glob = nc.dram_tensor("vals_glob", [NCORES * PADSH, 1], F32,
                               kind="ExternalInput")
    base_pat = nc.dram_tensor("base_pat", [128, NCAND], F32,
                              kind="ExternalInput")
    rowbase = nc.dram_tensor("rowbase", [128, 1], F32, kind="ExternalInput")
    y = nc.dram_tensor("y", [256, 1], F32, kind="ExternalOutput")

    with TileContext(nc) as tc:
        with tc.tile_pool(name="const", bufs=1) as cpool, \
             tc.tile_pool(name="work", bufs=6) as spool, \
             tc.tile_pool(name="cand", bufs=3) as candp, \
             tc.tile_pool(name="merge", bufs=1) as mpool, \
             tc.tile_pool(name="psum", bufs=6, space="PSUM") as psum, \
             tc.tile_pool(name="psum1", bufs=2, space="PSUM") as psum1, \
             tc.tile_pool(name="dram", bufs=1, space="DRAM") as dram:

            # ---------------- setup ----------------
            kT_t = cpool.tile([D, PADSH], F32)
            nc.sync.dma_start(out=kT_t[:], in_=kT[:])
            q2T = cpool.tile([D, B], F32)
            nc.sync.dma_start(out=q2T[:], in_=qT[:])
            nc.vector.tensor_scalar_mul(q2T[:], q2T[:], 2.0)

            ones1 = cpool.tile([1, 128], F32)
            nc.vector.memset(ones1[:], 1.0)
            negones = cpool.tile([128, 1], F32)
            nc.vector.memset(negones[:], -1.0)

            ksqn = cpool.tile([1, PADSH], F32)    # -||k||^2 row
            for c in range(NCH):
                sl = slice(c * CHUNK, (c + 1) * CHUNK)
                kk = spool.tile([D, CHUNK], F32, tag="kk")
                nc.vector.tensor_tensor(out=kk[:], in0=kT_t[:, sl],
                                        in1=kT_t[:, sl], op=ALU.mult)
                pk = psum1.tile([1, CHUNK], F32)
                nc.tensor.matmul(out=pk[:], lhsT=negones[:], rhs=kk[:],
                                 start=True, stop=True)
                nc.scalar.activation(out=ksqn[:, sl], in_=pk[:], func=AF.Copy)

            qsq_d = cpool.tile([128, 2], F32)
            for h in range(2):
                qr = spool.tile([128, D], F32, tag="qr")
                nc.sync.dma_start(out=qr[:], in_=qrows[h * 128:(h + 1) * 128, :])
                nc.scalar.activation(out=qr[:], in_=qr[:], func=AF.Square,
                                     accum_out=qsq_d[:, h:h + 1])
            nc.vector.tensor_scalar_add(qsq_d[:], qsq_d[:], DELTA)

            basep = cpool.tile([128, NCAND], F32)
            nc.sync.dma_start(out=basep[:], in_=base_pat[:])
            rowb = cpool.tile([128, 1], F32)
            nc.sync.dma_start(out=rowb[:], in_=rowbase[:])

            ag_in = dram.tile([B, 2 * NCAND], F32)
            ag_out = dram.tile([B, 2 * NCAND], F32)
            gidx_flat = dram.tile([128 * NMERGE, 1], F32)

            # ---------------- scan ----------------
            for qt in range(QT):
                cs = candp.tile([128, NCAND], F32, tag="cs")
                cp = candp.tile([128, NCAND], U16, tag="cp")
                for c in range(NCH):
                    p = psum.tile([128, CHUNK], F32)
                    nc.tensor.matmul(
                        out=p[:], lhsT=q2T[:, qt * 128:(qt + 1) * 128],
                        rhs=kT_t[:, c * CHUNK:(c + 1) * CHUNK],
                        start=True, stop=False)
                    nc.tensor.matmul(
                        out=p[:], lhsT=ones1[:],
                        rhs=ksqn[:, c * CHUNK:(c + 1) * CHUNK],
                        start=False, stop=True)
                    st = spool.tile([128, CHUNK], F32, tag="evict")
                    nc.scalar.activation(out=st[:], in_=p[:], func=AF.Copy)
                    nc.vector.max(out=cs[:, c * 8:(c + 1) * 8], in_=st[:])
                    nc.vector.max_index(out=cp[:, c * 8:(c + 1) * 8],
                                        in_max=cs[:, c * 8:(c + 1) * 8],
                                        in_values=st[:])
                gi = candp.tile([128, NCAND], F32, tag="gi")
                nc.vector.tensor_copy(gi[:], cp[:])       # u16 -> f32
                nc.vector.tensor_add(gi[:], gi[:], basep[:])
                # stage into AllToAll input: block j=(qt%8), half (qt//8)
                r0 = (qt % 8) * 256 + (qt // 8) * 128
                nc.sync.dma_start(out=ag_in[r0:r0 + 128, 0:NCAND], in_=cs[:])
                nc.sync.dma_start(out=ag_in[r0:r0 + 128, NCAND:2 * NCAND],
                                  in_=gi[:])

            nc.gpsimd.collective_compute(
                "AllToAll", ALU.bypass,
                replica_groups=[list(range(NCORES))],
                ins=[ag_in.opt()], outs=[ag_out.opt()],
            )

            # ---------------- merge (2 halves of 128 queries) -------------
            for m in range(2):
                # gather [128, 8 blocks x 200] from ag_out
                src = ag_out[:].rearrange("(blk p) c -> p blk c", p=256)
                smg = mpool.tile([128, NMERGE], F32, tag=f"smg{m}")
                gmg = mpool.tile([128, NMERGE], F32, tag=f"gmg{m}")
                off = m * 128
                nc.sync.dma_start(
                    out=smg[:].rearrange("p (blk j) -> p blk j", blk=NCORES),
                    in_=src[off:off + 128, :, 0:NCAND])
                nc.sync.dma_start(
                    out=gmg[:].rearrange("p (blk j) -> p blk j", blk=NCORES),
                    in_=src[off:off + 128, :, NCAND:2 * NCAND])
                # stage gidx (p-major flat) for slot-indexed row gather
                nc.sync.dma_start(
                    out=gidx_flat[:].rearrange("(p s) one -> p (s one)", p=128),
                    in_=gmg[:])

                work = mpool.tile([128, NMERGE], F32, tag=f"wk{m}")
                nc.vector.tensor_copy(work[:], smg[:])
                sel = mpool.tile([128, NSEL], F32, tag=f"sel{m}")
                slot = mpool.tile([128, NSEL], U32, tag=f"slot{m}")
                for r in range(NROUND):
                    s8 = sel[:, r * 8:(r + 1) * 8]
                    nc.vector.max(out=s8, in_=work[:])
                    nc.vector.max_index(out=slot[:, r * 8:(r + 1) * 8],
                                        in_max=s8, in_values=work[:])
                    if r + 1 < NROUND:
                        nc.vector.match_replace(out=work[:], in_to_replace=s8,
                                                in_values=work[:],
                                                imm_value=NEG)

                # winner rows in gidx_flat: row = p*1600 + slot
                slotf = mpool.tile([128, NSEL], F32, tag=f"slotf{m}")
                nc.vector.tensor_copy(slotf[:], slot[:])   # u32 -> f32
                nc.vector.tensor_scalar(slotf[:], slotf[:], rowb[:, 0:1], None,
                                        op0=ALU.add)
                rowf = mpool.tile([128, NSEL], U32, tag=f"rowf{m}")
                nc.vector.tensor_copy(rowf[:], slotf[:])   # f32 -> u32
                giw = mpool.tile([128, NSEL], F32, tag=f"giw{m}")
                for j in range(NSEL):
                    nc.gpsimd.indirect_dma_start(
                        out=giw[:, j:j + 1], out_offset=None,
                        in_=gidx_flat[:],
                        in_offset=bass.IndirectOffsetOnAxis(
                            ap=rowf[:, j:j + 1], axis=0))
                giwu = mpool.tile([128, NSEL], U32, tag=f"giwu{m}")
                nc.vector.tensor_copy(giwu[:], giw[:])    # f32 -> u32
                vw = mpool.tile([128, NSEL], F32, tag=f"vw{m}")
                for j in range(NSEL):
                    nc.gpsimd.indirect_dma_start(
                        out=vw[:, j:j + 1], out_offset=None,
                        in_=vals_glob[:],
                        in_offset=bass.IndirectOffsetOnAxis(
                            ap=giwu[:, j:j + 1], axis=0))

                # weights and output
                dd = mpool.tile([128, NSEL], F32, tag=f"dd{m}")
                nc.vector.tensor_scalar(dd[:], sel[:], qsq_d[:, m:m + 1], -1.0,
                                        op0=ALU.subtract, op1=ALU.mult)
                w = mpool.tile([128, NSEL], F32, tag=f"w{m}")
                nc.vector.reciprocal(w[:], dd[:])
                msk = mpool.tile([128, NSEL], F32, tag=f"msk{m}")
                nc.vector.tensor_scalar(msk[:], sel[:], sel[:, K - 1:K], None,
                                        op0=ALU.is_ge)
                nc.vector.tensor_tensor(out=w[:], in0=w[:], in1=msk[:],
                                        op=ALU.mult)
                wv = mpool.tile([128, NSEL], F32, tag=f"wv{m}")
                nc.vector.tensor_tensor(out=wv[:], in0=w[:], in1=vw[:],
                                        op=ALU.mult)
                num = mpool.tile([128, 1], F32, tag=f"num{m}")
                den = mpool.tile([128, 1], F32, tag=f"den{m}")
                nc.vector.reduce_sum(out=num[:], in_=wv[:],
                                     axis=mybir.AxisListType.X)
                nc.vector.reduce_sum(out=den[:], in_=w[:],
                                     axis=mybir.AxisListType.X)
                nc.vector.reciprocal(den[:], den[:])
                nc.vector.tensor_tensor(out=num[:], in0=num[:], in1=den[:],
                                        op=ALU.mult)
                nc.sync.dma_start(out=y[m * 128:(m + 1) * 128, :], in_=num[:])

    nc.compile()
    return nc


class _Runner:
    """Stable jitted PJRT runner (jit built once -> warm calls are fast)."""

    def __init__(self, nc, n_cores):
        from jax.sharding import Mesh, PartitionSpec
        from jax.experimental.shard_map import shard_map
        from concourse.bass2jax import (_bass_exec_p, install_neuronx_cc_hook,
                                        partition_id_tensor)
        install_neuronx_cc_hook()
        self.n_cores = n_cores
        pname = nc.partition_id_tensor.name if nc.partition_id_tensor else None
        in_names, out_names, out_avals, zero_outs = [], [], [], []
        for alloc in nc.m.functions[0].allocations:
            if not isinstance(alloc, mybir.MemoryLocationSet):
                continue
            name = alloc.memorylocations[0].name
            if alloc.kind == "ExternalInput":
                if name != pname:
                    in_names.append(name)
            elif alloc.kind == "ExternalOutput":
                out_names.append(name)
                shape = tuple(alloc.tensor_shape)
                dtype = mybir.dt.np(alloc.dtype)
                out_avals.append(jax.core.ShapedArray(shape, dtype))
                zero_outs.append(np.zeros(shape, dtype))
        self.in_names = in_names
        self.out_names = out_names
        self.zero_outs = zero_outs
        n_params, n_outs = len(in_names), len(out_names)
        all_in = in_names + out_names + ([pname] if pname else [])

        def _body(*args):
            operands = list(args)
            if pname is not None:
                operands.append(partition_id_tensor())
            return tuple(_bass_exec_p.bind(
                *operands, out_avals=tuple(out_avals), in_names=tuple(all_in),
                out_names=tuple(out_names), lowering_input_output_aliases=(),
                sim_require_finite=True, sim_require_nnan=True, nc=nc))

        devices = jax.devices()[:n_cores]
        mesh = Mesh(np.asarray(devices), ("core",))
        self.fn = jax.jit(shard_map(
            _body, mesh=mesh,
            in_specs=(PartitionSpec("core"),) * (n_params + n_outs),
            out_specs=(PartitionSpec("core"),) * n_outs, check_rep=False))

    def __call__(self, in_maps):
        args = [np.concatenate([np.asarray(m[n]) for m in in_maps], axis=0)
                for n in self.in_names]
        args += [np.concatenate([z] * self.n_cores, axis=0)
                 for z in self.zero_outs]
        outs = [np.asarray(o) for o in self.fn(*args)]
        res = []
        for c in range(self.n_cores):
            d = {}
            for n, o in zip(self.out_names, outs):
                per = o.shape[0] // self.n_cores
                d[n] = o[c * per:(c + 1) * per]
            res.append(d)
        return res


_CACHE = {}


def _prep_inputs(queries, dnd_keys, dnd_values):
    """Shard / lay out the inputs for the 8 cores (host-side data movement)."""
    q = np.ascontiguousarray(queries, dtype=np.float32)
    kk = np.ascontiguousarray(dnd_keys, dtype=np.float32)
    vv = np.ascontiguousarray(dnd_values, dtype=np.float32).reshape(-1)

    qT = np.ascontiguousarray(q.T)                       # [128, 2048]
    vals_glob = np.zeros((NCORES * PADSH, 1), np.float32)
    in_maps = []
    for c in range(NCORES):
        ksl = kk[c * SHARD:(c + 1) * SHARD]              # [12500, 128]
        kT_p = np.full((D, PADSH), PAD_KEY, np.float32)
        kT_p[:, :SHARD] = ksl.T
        vals_glob[c * PADSH:c * PADSH + SHARD, 0] = vv[c * SHARD:(c + 1) * SHARD]
        base = (np.arange(NCAND, dtype=np.float32) // 8).astype(np.float32) \
            * CHUNK + c * PADSH
        base_pat = np.broadcast_to(base, (128, NCAND)).astype(np.float32)
        qrows_c = np.concatenate([q[c * 128:(c + 1) * 128],
                                  q[(c + 8) * 128:(c + 9) * 128]], axis=0)
        rowbase = (np.arange(128, dtype=np.float32) * NMERGE)[:, None]
        in_maps.append({
            "kT": kT_p, "qT": qT, "qrows": qrows_c,
            "base_pat": np.ascontiguousarray(base_pat),
            "rowbase": np.ascontiguousarray(rowbase),
        })
    for c in range(NCORES):
        in_maps[c]["vals_glob"] = vals_glob
    return in_maps


def kernel(queries, dnd_keys, dnd_values):
    if "run" not in _CACHE:
        nc = _build()
        _CACHE["run"] = _Runner(nc, NCORES)
    run = _CACHE["run"]
    in_maps = _prep_inputs(queries, dnd_keys, dnd_values)
    results = run(in_maps)
    out = np.zeros((B, 1), np.float32)
    for c in range(NCORES):
        yc = results[c]["y"]
        out[c * 128:(c + 1) * 128] = yc[0:128]
        out[(c + 8) * 128:(c + 9) * 128] = yc[128:256]
    return out
